# revision 16
# baseline (speedup 1.0000x reference)
"""Trainium2 Bass kernel for BiLinearSigmoidAttention (length-sparse, bf16).

Reference math (per batch b, with L = length[b]):
    qn = l2norm(query), cn = l2norm(context)
    raw[q,k] = qn[q] . cn[k]            (masked: k >= L -> -1e30)
    sig = sigmoid(raw)
    den[q] = max(sum_k sig[q,k], 1)
    scores[q,k] = sig[q,k] / den[q]     (rows q >= L zeroed)
    att[q,:] = sum_k scores[q,k] * context[k,:]
    out = concat([qn, att], -1)
returns (out [B,S,2D], scores [B,S,S])

Key structure (8 NeuronCores, data parallel over B=32 -> 4 slots per core):
  - sigmoid(-1e30) == 0, so only the first T_b = ceil(L_b/128) row/col
    tile-blocks of the [S,S] score matrix are nonzero. Batches are sorted
    by T descending and dealt round-robin to cores; slot j of every core
    runs with the baked tile count ts[j] = max T in that deal group.
    Zero regions are DMA'd from a zeroed SBUF tile during each slot's
    compute phase.
  - all matmuls run in bf16 (tolerance is 2e-2); outputs are written bf16
    and upcast to fp32 on the host.
  - ALL transposes go through the DMA xbar (dma_start_transpose,
    extra-major row mapping out[p,e,l] = in_T[e*128+p, l]), so the PE
    only runs mm1 + attended matmuls. sg is stored [k', qb, kt, q_local]
    so one xbar transfer per q-block yields the scores row block directly.
  - scalar-engine activation functions are grouped (Square/Sqrt block,
    then Sigmoid block, then table-free Copies) because Sigmoid <->
    Square/Sqrt transitions cost a ~1.3us activation-table reload.
  - mm1 computes sigT [k_part, q_free]; the length mask is a per-partition
    bias and the context l2-norm a per-partition scale fused into the
    sigmoid activation. ps1 holds 4 PSUM banks so matmuls run ahead of
    the norm-gated sigmoid evictions.
  - DMA dispatch is spread over three queues: inputs + transposes on
    sync, ao on scalar (HWDGE), qn/score rows/zero fills on gpsimd.
"""

import numpy as np
import ml_dtypes

import concourse.bacc as bacc
import concourse.mybir as mybir
import concourse.tile as tile
from concourse.bass_utils import run_bass_kernel_spmd

B, S, D = 32, 1024, 512
NCORES = 8
BPC = B // NCORES          # batch slots per core
P = 128                    # partitions
NT = S // P                # 8 s-tiles
ND = D // P                # 4 d-chunks
NEG = np.float32(-1e30)

F32 = mybir.dt.float32
BF16 = mybir.dt.bfloat16
AF = mybir.ActivationFunctionType
ALU = mybir.AluOpType
AX = mybir.AxisListType


def build_kernel(ts):
    """ts: per-slot baked tile counts (len BPC, descending, each 1..NT)."""
    nc = bacc.Bacc("TRN2", target_bir_lowering=False, debug=False)

    q_d = nc.dram_tensor("query", [BPC, S, D], F32, kind="ExternalInput")
    c_d = nc.dram_tensor("context", [BPC, S, D], F32, kind="ExternalInput")
    # masks[b, p, kt]      = 0 if kt*P+p < L else -1e30   (cols 0..NT)
    # masks[b, p, NT + qb] = 1 if qb*P+p < L else 0       (cols NT..2NT)
    mk_d = nc.dram_tensor("masks", [BPC, P, 2 * NT], F32, kind="ExternalInput")
    out_d = nc.dram_tensor("out", [BPC, S, 2 * D], BF16, kind="ExternalOutput")
    sc_d = nc.dram_tensor("scores", [BPC, S, S], BF16, kind="ExternalOutput")

    with tile.TileContext(nc) as tc:
        _body(tc, ts, q_d, c_d, mk_d, out_d, sc_d)
    nc.compile()
    return nc


def _body(tc, ts, q_d, c_d, mk_d, out_d, sc_d):
    nc = tc.nc
    from contextlib import ExitStack

    ctx = ExitStack()
    with ctx:
        const = ctx.enter_context(tc.tile_pool(name="const", bufs=1))
        qpool = ctx.enter_context(tc.tile_pool(name="q", bufs=2))
        cpool = ctx.enter_context(tc.tile_pool(name="c", bufs=2))
        qbp = ctx.enter_context(tc.tile_pool(name="qb", bufs=2))
        cbp = ctx.enter_context(tc.tile_pool(name="cb", bufs=2))
        tp = ctx.enter_context(tc.tile_pool(name="t", bufs=2))
        sgp = ctx.enter_context(tc.tile_pool(name="sg", bufs=2))
        mpool = ctx.enter_context(tc.tile_pool(name="m", bufs=2))
        spool = ctx.enter_context(tc.tile_pool(name="s", bufs=3))
        opool = ctx.enter_context(tc.tile_pool(name="o", bufs=3))
        ps1 = ctx.enter_context(tc.tile_pool(name="ps1", bufs=4, space="PSUM"))
        ps2 = ctx.enter_context(tc.tile_pool(name="ps2", bufs=4, space="PSUM"))

        zt = const.tile([P, S], BF16, tag="zt")
        nc.gpsimd.memset(zt[:], 0.0)

        for b in range(BPC):
            T = ts[b]
            W = T * P                      # active score width
            NQC = (W + 511) // 512         # 512-col q chunks for mm1

            # ssq/nrm/inv column layout: [c0 | q0..q3 | c1..cT-1 | q4..q7]
            # matches input arrival order so each half gets one Sqrt.
            def ccol(kt):
                return 0 if kt == 0 else 4 + kt

            def qcol(t):
                return 1 + t if t < 4 else T + t

            NC_ = NT + T                   # total norm columns

            # ---- input DMAs: c0 | q0-3 | c-rest | q4-7 | masks (sync) ----
            qt_t = qpool.tile([P, NT, D], F32, tag="qt")
            ct_t = cpool.tile([P, NT, D], F32, tag="ct")
            mk = mpool.tile([P, 2 * NT], F32, tag="mk")
            nc.sync.dma_start(ct_t[:, 0], c_d[b, 0:P, :])
            nc.sync.dma_start(
                qt_t[:, 0:4], q_d[b, 0:512, :].rearrange("(t p) d -> p t d", p=P)
            )
            if T > 1:
                nc.sync.dma_start(
                    ct_t[:, 1:T],
                    c_d[b, P:W, :].rearrange("(t p) d -> p t d", p=P),
                )
            nc.sync.dma_start(
                qt_t[:, 4:NT],
                q_d[b, 512:S, :].rearrange("(t p) d -> p t d", p=P),
            )
            nc.sync.dma_start(mk[:], mk_d[b])

            ssq = mpool.tile([P, NT + NT], F32, tag="ssq")
            nrm = mpool.tile([P, NT + NT], F32, tag="nrm")
            inv = mpool.tile([P, NT + NT], F32, tag="inv")
            qnb = qbp.tile([P, NT, D], BF16, tag="qnb")
            cbt = cbp.tile([P, NT, D], BF16, tag="cbt")
            qT = tp.tile([P, ND, W], BF16, tag="qT")
            cT = tp.tile([P, ND, W], BF16, tag="cT")
            # sg[k', qb, kt, q_local]: one xbar transfer per qb gives the
            # scores row block [q, kt*P + k'] directly.
            sg = sgp.tile([P, T, T, P], BF16, tag="sg")

            def square(col, src):
                scr2 = spool.tile([P, D], BF16, tag="scr2")
                nc.scalar.activation(
                    scr2[:], src, AF.Square, accum_out=ssq[:, col : col + 1]
                )

            # ---- first half: c0 + q0..q3 norms (scalar Square block) ----
            square(ccol(0), ct_t[:, 0])
            for t in range(4):
                square(qcol(t), qt_t[:, t])
            nc.scalar.activation(nrm[:, 0:5], ssq[:, 0:5], AF.Sqrt)
            nc.vector.reciprocal(inv[:, 0:5], nrm[:, 0:5])

            nc.scalar.copy(cbt[:, 0], ct_t[:, 0])
            nc.sync.dma_start_transpose(cT[:, :, 0:P], cbt[:, 0])
            for t in range(4):
                nc.vector.tensor_scalar_mul(
                    qnb[:, t], qt_t[:, t], inv[:, qcol(t) : qcol(t) + 1]
                )
                if t < T:
                    nc.sync.dma_start_transpose(
                        qT[:, :, t * P : (t + 1) * P], qnb[:, t]
                    )

            # ---- second half: c1..cT-1 + q4..q7 norms ----
            for kt in range(1, T):
                square(ccol(kt), ct_t[:, kt])
            for t in range(4, NT):
                square(qcol(t), qt_t[:, t])
            if NC_ > 5:
                nc.scalar.activation(nrm[:, 5:NC_], ssq[:, 5:NC_], AF.Sqrt)
                nc.vector.reciprocal(inv[:, 5:NC_], nrm[:, 5:NC_])

            for kt in range(1, T):
                nc.scalar.copy(cbt[:, kt], ct_t[:, kt])
                nc.sync.dma_start_transpose(
                    cT[:, :, kt * P : (kt + 1) * P], cbt[:, kt]
                )
            for t in range(4, NT):
                nc.vector.tensor_scalar_mul(
                    qnb[:, t], qt_t[:, t], inv[:, qcol(t) : qcol(t) + 1]
                )
                if t < T:
                    nc.sync.dma_start_transpose(
                        qT[:, :, t * P : (t + 1) * P], qnb[:, t]
                    )

            nc.gpsimd.dma_start(
                out_d[b, :, 0:D].rearrange("(t p) d -> p t d", p=P), qnb[:]
            )

            # ---- zero fills for this slot (execute during compute) ----
            for qt in range(T, NT):
                nc.gpsimd.dma_start(sc_d[b, qt * P : (qt + 1) * P, :], zt[:])
                nc.gpsimd.dma_start(
                    out_d[b, qt * P : (qt + 1) * P, D : 2 * D], zt[:, 0:D]
                )

            # ---- mm1: sigT[k, q] = sigmoid(inv_c[k] * (cT.T @ qT) + mask) ----
            for qc in range(NQC):
                wq = min(512, W - qc * 512)
                nqb = wq // P
                for kt in range(T):
                    acc = ps1.tile([P, 512], F32, tag="acc")
                    for dch in range(ND):
                        nc.tensor.matmul(
                            acc[:, 0:wq],
                            cT[:, dch, kt * P : (kt + 1) * P],
                            qT[:, dch, qc * 512 : qc * 512 + wq],
                            start=(dch == 0),
                            stop=(dch == ND - 1),
                        )
                    nc.scalar.activation(
                        sg[:, qc * 4 : qc * 4 + nqb, kt, :], acc[:, 0:wq],
                        AF.Sigmoid, bias=mk[:, kt : kt + 1],
                        scale=inv[:, ccol(kt) : ccol(kt) + 1],
                    )

            # ---- per q-block: xbar-transpose scores, den, w, attended ----
            aob = qbp.tile([P, T, D], BF16, tag="aob")
            for qb in range(T):
                so = opool.tile([P, T, P], BF16, tag="so")
                if W < S:
                    nc.gpsimd.dma_start(
                        sc_d[b, qb * P : (qb + 1) * P, W:S], zt[:, 0 : S - W]
                    )
                nc.sync.dma_start_transpose(so[:], sg[:, qb])

                att = ps2.tile([P, 512], F32, tag="att")
                for kt in range(T):
                    nc.tensor.matmul(
                        att[:], sg[:, qb, kt, :], cbt[:, kt],
                        start=(kt == 0), stop=(kt == T - 1),
                    )

                # w = qmask / max(den, 1)
                den = mpool.tile([P, 1], F32, tag="den")
                w = mpool.tile([P, 1], F32, tag="w")
                nc.vector.reduce_sum(den[:], so[:], axis=AX.XY)
                nc.vector.tensor_scalar_max(den[:], den[:], 1.0)
                nc.vector.reciprocal(w[:], den[:])
                nc.vector.tensor_mul(w[:], w[:], mk[:, NT + qb : NT + qb + 1])

                # scale scores in place, write out
                nc.vector.tensor_scalar_mul(so[:], so[:], w[:])
                nc.gpsimd.dma_start(sc_d[b, qb * P : (qb + 1) * P, 0:W], so[:])

                nc.vector.tensor_scalar_mul(aob[:, qb], att[:], w[:])

            nc.scalar.dma_start(
                out_d[b, 0:W, D : 2 * D].rearrange("(t p) d -> p t d", p=P),
                aob[:],
            )


_NC_CACHE = {}


def _get_nc(ts):
    key = ("nc", ts)
    if key not in _NC_CACHE:
        _NC_CACHE[key] = build_kernel(ts)
    return _NC_CACHE[key]


def plan(length):
    """Sort batches by tile count desc, deal round-robin to cores.

    Returns (ts, order): ts[j] = baked tile count for slot j; order[j*NCORES+c]
    = batch index placed in slot j of core c.
    """
    length = np.asarray(length).astype(np.int64)
    T = np.ceil(length / P).astype(np.int64)
    order = np.argsort(-T, kind="stable")
    ts = tuple(int(T[order[j * NCORES]]) for j in range(BPC))
    return ts, order


def prep_inputs(context, query, length):
    context = np.ascontiguousarray(np.asarray(context, dtype=np.float32))
    query = np.ascontiguousarray(np.asarray(query, dtype=np.float32))
    length = np.asarray(length).astype(np.int64)
    ts, order = plan(length)

    iot = np.arange(S)
    keymask = iot[None, :] < length[:, None]                      # [B, S]
    kbH = np.where(keymask, np.float32(0.0), NEG).astype(np.float32)
    kbH = kbH.reshape(B, NT, P).transpose(0, 2, 1)
    qmH = keymask.astype(np.float32).reshape(B, NT, P).transpose(0, 2, 1)
    mkH = np.ascontiguousarray(np.concatenate([kbH, qmH], axis=2))

    in_maps = []
    for c in range(NCORES):
        bidx = [int(order[j * NCORES + c]) for j in range(BPC)]
        in_maps.append(
            {
                "query": np.ascontiguousarray(query[bidx]),
                "context": np.ascontiguousarray(context[bidx]),
                "masks": np.ascontiguousarray(mkH[bidx]),
            }
        )
    return ts, order, in_maps


def kernel(context, query, length):
    ts, order, in_maps = prep_inputs(context, query, length)
    nc = _get_nc(ts)
    res = run_bass_kernel_spmd(nc, in_maps, list(range(NCORES)))
    _NC_CACHE["last_result"] = res

    out = np.empty((B, S, 2 * D), np.float32)
    scores = np.empty((B, S, S), np.float32)
    for c in range(NCORES):
        ro = np.asarray(res.results[c]["out"]).astype(np.float32)
        rs = np.asarray(res.results[c]["scores"]).astype(np.float32)
        for j in range(BPC):
            bi = int(order[j * NCORES + c])
            out[bi] = ro[j]
            scores[bi] = rs[j]
    return out, scores


# revision 17
# speedup vs baseline: 1.5070x; 1.5070x over previous
"""Trainium2 Bass kernel for BiLinearSigmoidAttention (length-sparse, bf16).

Reference math (per batch b, with L = length[b]):
    qn = l2norm(query), cn = l2norm(context)
    raw[q,k] = qn[q] . cn[k]            (masked: k >= L -> -1e30)
    sig = sigmoid(raw)
    den[q] = max(sum_k sig[q,k], 1)
    scores[q,k] = sig[q,k] / den[q]     (rows q >= L zeroed)
    att[q,:] = sum_k scores[q,k] * context[k,:]
    out = concat([qn, att], -1)
returns (out [B,S,2D], scores [B,S,S])

Key structure (8 NeuronCores, data parallel over B=32 -> 4 slots per core):
  - sigmoid(-1e30) == 0, so only the first T_b = ceil(L_b/128) row/col
    tile-blocks of the [S,S] score matrix are nonzero. Batches are sorted
    by T descending and dealt round-robin to cores; slot j of every core
    runs with the baked tile count ts[j] = max T in that deal group.
    Zero regions are DMA'd from a zeroed SBUF tile during compute.
  - all matmuls and PE transposes run in bf16 (tolerance is 2e-2);
    outputs are written bf16 and upcast to fp32 on the host.
  - emission is software-pipelined: slot b+1's input DMAs and front-half
    compute are emitted before slot b's per-q-block phase, so input
    streaming and PE work never starve at slot boundaries.
  - qT/cT transposes run on the PE (cheap in bf16); the per-q-block score
    transpose uses one DMA-xbar transfer (dma_start_transpose, extra-major
    row mapping) per block: sg is stored [k', qb, kt, q_local] so the
    transfer yields the scores row block directly, PE runs only matmuls
    in the back half.
  - scalar activation functions are grouped (Square/Sqrt, then Sigmoid,
    then table-free Copies): Sigmoid <-> Square/Sqrt transitions cost a
    ~1.3us activation-table reload.
  - mm1 computes sigT [k_part, q_free]; the length mask is a per-partition
    bias and the context l2-norm a per-partition scale fused into the
    sigmoid activation; ps1 holds 4 PSUM banks so matmuls run ahead of
    the norm-gated sigmoid evictions.
  - DMA dispatch spread over three queues: inputs + score xbar on sync,
    ao on scalar (HWDGE), qn/score rows/zero fills on gpsimd (SWDGE).
"""

import numpy as np
import ml_dtypes

import concourse.bacc as bacc
import concourse.mybir as mybir
import concourse.tile as tile
from concourse.bass_utils import run_bass_kernel_spmd

B, S, D = 32, 1024, 512
NCORES = 8
BPC = B // NCORES          # batch slots per core
P = 128                    # partitions
NT = S // P                # 8 s-tiles
ND = D // P                # 4 d-chunks
NEG = np.float32(-1e30)

F32 = mybir.dt.float32
BF16 = mybir.dt.bfloat16
AF = mybir.ActivationFunctionType
ALU = mybir.AluOpType
AX = mybir.AxisListType


def build_kernel(ts):
    """ts: per-slot baked tile counts (len BPC, descending, each 1..NT)."""
    nc = bacc.Bacc("TRN2", target_bir_lowering=False, debug=False)

    q_d = nc.dram_tensor("query", [BPC, S, D], F32, kind="ExternalInput")
    c_d = nc.dram_tensor("context", [BPC, S, D], F32, kind="ExternalInput")
    # masks[b, p, kt]      = 0 if kt*P+p < L else -1e30   (cols 0..NT)
    # masks[b, p, NT + qb] = 1 if qb*P+p < L else 0       (cols NT..2NT)
    mk_d = nc.dram_tensor("masks", [BPC, P, 2 * NT], F32, kind="ExternalInput")
    id_d = nc.dram_tensor("identity", [P, P], BF16, kind="ExternalInput")
    out_d = nc.dram_tensor("out", [BPC, S, 2 * D], BF16, kind="ExternalOutput")
    sc_d = nc.dram_tensor("scores", [BPC, S, S], BF16, kind="ExternalOutput")

    with tile.TileContext(nc) as tc:
        _body(tc, ts, q_d, c_d, mk_d, id_d, out_d, sc_d)
    nc.compile()
    return nc


def _body(tc, ts, q_d, c_d, mk_d, id_d, out_d, sc_d):
    nc = tc.nc
    from contextlib import ExitStack

    ctx = ExitStack()
    with ctx:
        const = ctx.enter_context(tc.tile_pool(name="const", bufs=1))
        qpool = ctx.enter_context(tc.tile_pool(name="q", bufs=2))
        cpool = ctx.enter_context(tc.tile_pool(name="c", bufs=2))
        qbp = ctx.enter_context(tc.tile_pool(name="qb", bufs=2))
        cbp = ctx.enter_context(tc.tile_pool(name="cb", bufs=2))
        tp = ctx.enter_context(tc.tile_pool(name="t", bufs=2))
        sgp = ctx.enter_context(tc.tile_pool(name="sg", bufs=2))
        mpool = ctx.enter_context(tc.tile_pool(name="m", bufs=2))
        spool = ctx.enter_context(tc.tile_pool(name="s", bufs=3))
        opool = ctx.enter_context(tc.tile_pool(name="o", bufs=3))
        ps1 = ctx.enter_context(tc.tile_pool(name="ps1", bufs=4, space="PSUM"))
        pst = ctx.enter_context(tc.tile_pool(name="pst", bufs=2, space="PSUM"))
        ps2 = ctx.enter_context(tc.tile_pool(name="ps2", bufs=2, space="PSUM"))

        idb = const.tile([P, P], BF16, tag="idb")
        nc.sync.dma_start(idb[:], id_d[:])
        zt = const.tile([P, S], BF16, tag="zt")
        nc.gpsimd.memset(zt[:], 0.0)

        slots = {}

        def phase1(b):
            """inputs, norms, qn/cbt, qT/cT transposes, mm1+sigmoid."""
            T = ts[b]
            W = T * P
            NQC = (W + 511) // 512

            def ccol(kt):
                return 0 if kt == 0 else 4 + kt

            def qcol(t):
                return 1 + t if t < 4 else T + t

            NC_ = NT + T

            qt_t = qpool.tile([P, NT, D], F32, tag="qt")
            ct_t = cpool.tile([P, NT, D], F32, tag="ct")
            mk = mpool.tile([P, 2 * NT], F32, tag="mk")
            nc.sync.dma_start(ct_t[:, 0], c_d[b, 0:P, :])
            nc.sync.dma_start(
                qt_t[:, 0:4], q_d[b, 0:512, :].rearrange("(t p) d -> p t d", p=P)
            )
            if T > 1:
                nc.sync.dma_start(
                    ct_t[:, 1:T],
                    c_d[b, P:W, :].rearrange("(t p) d -> p t d", p=P),
                )
            nc.sync.dma_start(
                qt_t[:, 4:NT],
                q_d[b, 512:S, :].rearrange("(t p) d -> p t d", p=P),
            )
            nc.sync.dma_start(mk[:], mk_d[b])

            ssq = mpool.tile([P, NT + NT], F32, tag="ssq")
            nrm = mpool.tile([P, NT + NT], F32, tag="nrm")
            inv = mpool.tile([P, NT + NT], F32, tag="inv")
            qnb = qbp.tile([P, NT, D], BF16, tag="qnb")
            cbt = cbp.tile([P, NT, D], BF16, tag="cbt")
            qT = tp.tile([P, ND, W], BF16, tag="qT")
            cT = tp.tile([P, ND, W], BF16, tag="cT")
            # sg[k', qb, kt, q_local]: one xbar transfer per qb gives the
            # scores row block [q, kt*P + k'] directly.
            sg = sgp.tile([P, T, T, P], BF16, tag="sg")

            def square(col, src):
                scr2 = spool.tile([P, D], BF16, tag="scr2")
                nc.scalar.activation(
                    scr2[:], src, AF.Square, accum_out=ssq[:, col : col + 1]
                )

            def transpose_tile(src, dst_T, t, evict_vec):
                pq = pst.tile([P, ND, P], BF16, tag="pt")
                for dch in range(ND):
                    nc.tensor.transpose(
                        pq[:, dch], src[:, dch * P : (dch + 1) * P], idb[:]
                    )
                if evict_vec:
                    nc.vector.tensor_copy(dst_T[:, :, t * P : (t + 1) * P], pq[:])
                else:
                    nc.scalar.copy(dst_T[:, :, t * P : (t + 1) * P], pq[:])

            # first half: c0 + q0..q3 norms (scalar Square/Sqrt block)
            square(ccol(0), ct_t[:, 0])
            for t in range(4):
                square(qcol(t), qt_t[:, t])
            nc.scalar.activation(nrm[:, 0:5], ssq[:, 0:5], AF.Sqrt)
            nc.vector.reciprocal(inv[:, 0:5], nrm[:, 0:5])

            nc.vector.tensor_copy(cbt[:, 0], ct_t[:, 0])
            for t in range(4):
                nc.vector.tensor_scalar_mul(
                    qnb[:, t], qt_t[:, t], inv[:, qcol(t) : qcol(t) + 1]
                )
            transpose_tile(cbt[:, 0], cT, 0, evict_vec=True)
            for t in range(min(4, T)):
                transpose_tile(qnb[:, t], qT, t, evict_vec=False)

            # second half: c1..cT-1 + q4..q7 norms
            for kt in range(1, T):
                square(ccol(kt), ct_t[:, kt])
            for t in range(4, NT):
                square(qcol(t), qt_t[:, t])
            if NC_ > 5:
                nc.scalar.activation(nrm[:, 5:NC_], ssq[:, 5:NC_], AF.Sqrt)
                nc.vector.reciprocal(inv[:, 5:NC_], nrm[:, 5:NC_])

            for kt in range(1, T):
                nc.vector.tensor_copy(cbt[:, kt], ct_t[:, kt])
                transpose_tile(cbt[:, kt], cT, kt, evict_vec=True)
            for t in range(4, NT):
                nc.vector.tensor_scalar_mul(
                    qnb[:, t], qt_t[:, t], inv[:, qcol(t) : qcol(t) + 1]
                )
                if t < T:
                    transpose_tile(qnb[:, t], qT, t, evict_vec=False)

            nc.gpsimd.dma_start(
                out_d[b, :, 0:D].rearrange("(t p) d -> p t d", p=P), qnb[:]
            )
            # zero fills for this slot (execute during compute)
            for qt in range(T, NT):
                nc.gpsimd.dma_start(sc_d[b, qt * P : (qt + 1) * P, :], zt[:])
                nc.gpsimd.dma_start(
                    out_d[b, qt * P : (qt + 1) * P, D : 2 * D], zt[:, 0:D]
                )

            # mm1: sigT[k, q] = sigmoid(inv_c[k] * (cT.T @ qT) + mask)
            for qc in range(NQC):
                wq = min(512, W - qc * 512)
                nqb = wq // P
                for kt in range(T):
                    acc = ps1.tile([P, 512], F32, tag="acc")
                    for dch in range(ND):
                        nc.tensor.matmul(
                            acc[:, 0:wq],
                            cT[:, dch, kt * P : (kt + 1) * P],
                            qT[:, dch, qc * 512 : qc * 512 + wq],
                            start=(dch == 0),
                            stop=(dch == ND - 1),
                        )
                    nc.scalar.activation(
                        sg[:, qc * 4 : qc * 4 + nqb, kt, :], acc[:, 0:wq],
                        AF.Sigmoid, bias=mk[:, kt : kt + 1],
                        scale=inv[:, ccol(kt) : ccol(kt) + 1],
                    )

            slots[b] = dict(T=T, W=W, mk=mk, cbt=cbt, sg=sg)

        def phase2(b):
            """per q-block: xbar score transpose, den, w, attended, writes."""
            st = slots.pop(b)
            T, W, mk, cbt, sg = st["T"], st["W"], st["mk"], st["cbt"], st["sg"]
            aob = qbp.tile([P, T, D], BF16, tag="aob")
            for qb in range(T):
                so = opool.tile([P, T, P], BF16, tag="so")
                if W < S:
                    nc.gpsimd.dma_start(
                        sc_d[b, qb * P : (qb + 1) * P, W:S], zt[:, 0 : S - W]
                    )
                nc.sync.dma_start_transpose(so[:], sg[:, qb])

                att = ps2.tile([P, 512], F32, tag="att")
                for kt in range(T):
                    nc.tensor.matmul(
                        att[:], sg[:, qb, kt, :], cbt[:, kt],
                        start=(kt == 0), stop=(kt == T - 1),
                    )

                # w = qmask / max(den, 1)
                den = mpool.tile([P, 1], F32, tag="den")
                w = mpool.tile([P, 1], F32, tag="w")
                nc.vector.reduce_sum(den[:], so[:], axis=AX.XY)
                nc.vector.tensor_scalar_max(den[:], den[:], 1.0)
                nc.vector.reciprocal(w[:], den[:])
                nc.vector.tensor_mul(w[:], w[:], mk[:, NT + qb : NT + qb + 1])

                # scale scores in place, write out
                nc.vector.tensor_scalar_mul(so[:], so[:], w[:])
                nc.gpsimd.dma_start(sc_d[b, qb * P : (qb + 1) * P, 0:W], so[:])

                nc.vector.tensor_scalar_mul(aob[:, qb], att[:], w[:])

            nc.scalar.dma_start(
                out_d[b, 0:W, D : 2 * D].rearrange("(t p) d -> p t d", p=P),
                aob[:],
            )

        # software-pipelined emission: slot b+1's front half is queued
        # before slot b's back half.
        phase1(0)
        for b in range(BPC):
            if b + 1 < BPC:
                phase1(b + 1)
            phase2(b)


_NC_CACHE = {}


def _get_nc(ts):
    key = ("nc", ts)
    if key not in _NC_CACHE:
        _NC_CACHE[key] = build_kernel(ts)
    return _NC_CACHE[key]


def plan(length):
    """Sort batches by tile count desc, deal round-robin to cores.

    Returns (ts, order): ts[j] = baked tile count for slot j; order[j*NCORES+c]
    = batch index placed in slot j of core c.
    """
    length = np.asarray(length).astype(np.int64)
    T = np.ceil(length / P).astype(np.int64)
    order = np.argsort(-T, kind="stable")
    ts = tuple(int(T[order[j * NCORES]]) for j in range(BPC))
    return ts, order


def prep_inputs(context, query, length):
    context = np.ascontiguousarray(np.asarray(context, dtype=np.float32))
    query = np.ascontiguousarray(np.asarray(query, dtype=np.float32))
    length = np.asarray(length).astype(np.int64)
    ts, order = plan(length)

    iot = np.arange(S)
    keymask = iot[None, :] < length[:, None]                      # [B, S]
    kbH = np.where(keymask, np.float32(0.0), NEG).astype(np.float32)
    kbH = kbH.reshape(B, NT, P).transpose(0, 2, 1)
    qmH = keymask.astype(np.float32).reshape(B, NT, P).transpose(0, 2, 1)
    mkH = np.ascontiguousarray(np.concatenate([kbH, qmH], axis=2))
    idb = np.eye(P, dtype=ml_dtypes.bfloat16)

    in_maps = []
    for c in range(NCORES):
        bidx = [int(order[j * NCORES + c]) for j in range(BPC)]
        in_maps.append(
            {
                "query": np.ascontiguousarray(query[bidx]),
                "context": np.ascontiguousarray(context[bidx]),
                "masks": np.ascontiguousarray(mkH[bidx]),
                "identity": idb,
            }
        )
    return ts, order, in_maps


def kernel(context, query, length):
    ts, order, in_maps = prep_inputs(context, query, length)
    nc = _get_nc(ts)
    res = run_bass_kernel_spmd(nc, in_maps, list(range(NCORES)))
    _NC_CACHE["last_result"] = res

    out = np.empty((B, S, 2 * D), np.float32)
    scores = np.empty((B, S, S), np.float32)
    for c in range(NCORES):
        ro = np.asarray(res.results[c]["out"]).astype(np.float32)
        rs = np.asarray(res.results[c]["scores"]).astype(np.float32)
        for j in range(BPC):
            bi = int(order[j * NCORES + c])
            out[bi] = ro[j]
            scores[bi] = rs[j]
    return out, scores


# revision 18
# speedup vs baseline: 1.6099x; 1.0683x over previous
"""Trainium2 Bass kernel for BiLinearSigmoidAttention (length-sparse, bf16).

Reference math (per batch b, with L = length[b]):
    qn = l2norm(query), cn = l2norm(context)
    raw[q,k] = qn[q] . cn[k]            (masked: k >= L -> -1e30)
    sig = sigmoid(raw)
    den[q] = max(sum_k sig[q,k], 1)
    scores[q,k] = sig[q,k] / den[q]     (rows q >= L zeroed)
    att[q,:] = sum_k scores[q,k] * context[k,:]
    out = concat([qn, att], -1)
returns (out [B,S,2D], scores [B,S,S])

Key structure (8 NeuronCores, data parallel over B=32 -> 4 slots per core):
  - sigmoid(-1e30) == 0, so only the first T_b = ceil(L_b/128) row/col
    tile-blocks of the [S,S] score matrix are nonzero. Batches are sorted
    by T descending and dealt round-robin to cores; slot j of every core
    runs with the baked tile count ts[j] = max T in that deal group.
    Zero regions are DMA'd from a zeroed SBUF tile during compute.
  - all matmuls and PE transposes run in bf16 (tolerance is 2e-2);
    outputs are written bf16 and upcast to fp32 on the host.
  - emission is software-pipelined: slot b+1's input DMAs and front-half
    compute are emitted before slot b's per-q-block phase, so input
    streaming and PE work never starve at slot boundaries.
  - qT/cT transposes run on the PE (cheap in bf16); the per-q-block score
    transpose uses one DMA-xbar transfer (dma_start_transpose, extra-major
    row mapping) per block: sg is stored [k', qb, kt, q_local] so the
    transfer yields the scores row block directly, PE runs only matmuls
    in the back half.
  - scalar activation functions are grouped (Square/Sqrt, then Sigmoid,
    then table-free Copies): Sigmoid <-> Square/Sqrt transitions cost a
    ~1.3us activation-table reload.
  - mm1 computes sigT [k_part, q_free]; the length mask is a per-partition
    bias and the context l2-norm a per-partition scale fused into the
    sigmoid activation; ps1 holds 4 PSUM banks so matmuls run ahead of
    the norm-gated sigmoid evictions.
  - DMA dispatch spread over three queues: inputs + score xbar on sync,
    ao on scalar (HWDGE), qn/score rows/zero fills on gpsimd (SWDGE).
"""

import numpy as np
import ml_dtypes

import concourse.bacc as bacc
import concourse.mybir as mybir
import concourse.tile as tile
from concourse.bass_utils import run_bass_kernel_spmd

B, S, D = 32, 1024, 512
NCORES = 8
BPC = B // NCORES          # batch slots per core
P = 128                    # partitions
NT = S // P                # 8 s-tiles
ND = D // P                # 4 d-chunks
NEG = np.float32(-1e30)

F32 = mybir.dt.float32
BF16 = mybir.dt.bfloat16
AF = mybir.ActivationFunctionType
ALU = mybir.AluOpType
AX = mybir.AxisListType


def build_kernel(ts):
    """ts: per-slot baked tile counts (len BPC, descending, each 1..NT)."""
    nc = bacc.Bacc("TRN2", target_bir_lowering=False, debug=False)

    q_d = nc.dram_tensor("query", [BPC, S, D], F32, kind="ExternalInput")
    c_d = nc.dram_tensor("context", [BPC, S, D], F32, kind="ExternalInput")
    # masks[b, p, kt]      = 0 if kt*P+p < L else -1e30   (cols 0..NT)
    # masks[b, p, NT + qb] = 1 if qb*P+p < L else 0       (cols NT..2NT)
    mk_d = nc.dram_tensor("masks", [BPC, P, 2 * NT], F32, kind="ExternalInput")
    id_d = nc.dram_tensor("identity", [P, P], BF16, kind="ExternalInput")
    out_d = nc.dram_tensor("out", [BPC, S, 2 * D], BF16, kind="ExternalOutput")
    sc_d = nc.dram_tensor("scores", [BPC, S, S], BF16, kind="ExternalOutput")

    with tile.TileContext(nc) as tc:
        _body(tc, ts, q_d, c_d, mk_d, id_d, out_d, sc_d)
    nc.compile()
    return nc


def _body(tc, ts, q_d, c_d, mk_d, id_d, out_d, sc_d):
    nc = tc.nc
    from contextlib import ExitStack

    ctx = ExitStack()
    with ctx:
        const = ctx.enter_context(tc.tile_pool(name="const", bufs=1))
        qpool = ctx.enter_context(tc.tile_pool(name="q", bufs=2))
        cpool = ctx.enter_context(tc.tile_pool(name="c", bufs=2))
        qbp = ctx.enter_context(tc.tile_pool(name="qb", bufs=2))
        cbp = ctx.enter_context(tc.tile_pool(name="cb", bufs=2))
        tp = ctx.enter_context(tc.tile_pool(name="t", bufs=2))
        sgp = ctx.enter_context(tc.tile_pool(name="sg", bufs=2))
        mpool = ctx.enter_context(tc.tile_pool(name="m", bufs=2))
        spool = ctx.enter_context(tc.tile_pool(name="s", bufs=3))
        opool = ctx.enter_context(tc.tile_pool(name="o", bufs=3))
        ps1 = ctx.enter_context(tc.tile_pool(name="ps1", bufs=4, space="PSUM"))
        pst = ctx.enter_context(tc.tile_pool(name="pst", bufs=2, space="PSUM"))
        ps2 = ctx.enter_context(tc.tile_pool(name="ps2", bufs=2, space="PSUM"))

        idb = const.tile([P, P], BF16, tag="idb")
        nc.sync.dma_start(idb[:], id_d[:])
        zt = const.tile([P, S], BF16, tag="zt")
        nc.gpsimd.memset(zt[:], 0.0)

        slots = {}

        def phase1(b):
            """inputs, norms, qn/cbt, qT/cT transposes, mm1+sigmoid."""
            T = ts[b]
            W = T * P
            NQC = (W + 511) // 512

            def ccol(kt):
                return 0 if kt == 0 else 4 + kt

            def qcol(t):
                return 1 + t if t < 4 else T + t

            NC_ = NT + T

            qt_t = qpool.tile([P, NT, D], F32, tag="qt")
            ct_t = cpool.tile([P, NT, D], F32, tag="ct")
            mk = mpool.tile([P, 2 * NT], F32, tag="mk")
            nc.sync.dma_start(ct_t[:, 0], c_d[b, 0:P, :])
            nc.sync.dma_start(
                qt_t[:, 0:4], q_d[b, 0:512, :].rearrange("(t p) d -> p t d", p=P)
            )
            if T > 1:
                nc.sync.dma_start(
                    ct_t[:, 1:T],
                    c_d[b, P:W, :].rearrange("(t p) d -> p t d", p=P),
                )
            nc.sync.dma_start(
                qt_t[:, 4:NT],
                q_d[b, 512:S, :].rearrange("(t p) d -> p t d", p=P),
            )
            nc.sync.dma_start(mk[:], mk_d[b])

            ssq = mpool.tile([P, NT + NT], F32, tag="ssq")
            nrm = mpool.tile([P, NT + NT], F32, tag="nrm")
            inv = mpool.tile([P, NT + NT], F32, tag="inv")
            qnb = qbp.tile([P, NT, D], BF16, tag="qnb")
            cbt = cbp.tile([P, NT, D], BF16, tag="cbt")
            qT = tp.tile([P, ND, W], BF16, tag="qT")
            cT = tp.tile([P, ND, W], BF16, tag="cT")
            # sg[k', qb, kt, q_local]: one xbar transfer per qb gives the
            # scores row block [q, kt*P + k'] directly.
            sg = sgp.tile([P, T, T, P], BF16, tag="sg")

            def square(col, src):
                scr2 = spool.tile([P, D], BF16, tag="scr2")
                nc.scalar.activation(
                    scr2[:], src, AF.Square, accum_out=ssq[:, col : col + 1]
                )

            def transpose_tile(src, dst_T, t, evict_vec):
                pq = pst.tile([P, ND, P], BF16, tag="pt")
                for dch in range(ND):
                    nc.tensor.transpose(
                        pq[:, dch], src[:, dch * P : (dch + 1) * P], idb[:]
                    )
                if evict_vec:
                    nc.vector.tensor_copy(dst_T[:, :, t * P : (t + 1) * P], pq[:])
                else:
                    nc.scalar.copy(dst_T[:, :, t * P : (t + 1) * P], pq[:])

            # first half: c0 + q0..q3 norms (scalar Square/Sqrt block)
            square(ccol(0), ct_t[:, 0])
            for t in range(4):
                square(qcol(t), qt_t[:, t])
            nc.scalar.activation(nrm[:, 0:5], ssq[:, 0:5], AF.Sqrt)
            nc.vector.reciprocal(inv[:, 0:5], nrm[:, 0:5])

            nc.vector.tensor_copy(cbt[:, 0], ct_t[:, 0])
            for t in range(4):
                nc.vector.tensor_scalar_mul(
                    qnb[:, t], qt_t[:, t], inv[:, qcol(t) : qcol(t) + 1]
                )
            transpose_tile(cbt[:, 0], cT, 0, evict_vec=True)
            for t in range(min(4, T)):
                transpose_tile(qnb[:, t], qT, t, evict_vec=False)

            # second half: c1..cT-1 + q4..q7 norms
            for kt in range(1, T):
                square(ccol(kt), ct_t[:, kt])
            for t in range(4, NT):
                square(qcol(t), qt_t[:, t])
            if NC_ > 5:
                nc.scalar.activation(nrm[:, 5:NC_], ssq[:, 5:NC_], AF.Sqrt)
                nc.vector.reciprocal(inv[:, 5:NC_], nrm[:, 5:NC_])

            for kt in range(1, T):
                nc.vector.tensor_copy(cbt[:, kt], ct_t[:, kt])
                transpose_tile(cbt[:, kt], cT, kt, evict_vec=True)
            for t in range(4, NT):
                nc.vector.tensor_scalar_mul(
                    qnb[:, t], qt_t[:, t], inv[:, qcol(t) : qcol(t) + 1]
                )
                if t < T:
                    transpose_tile(qnb[:, t], qT, t, evict_vec=False)

            nc.gpsimd.dma_start(
                out_d[b, :, 0:D].rearrange("(t p) d -> p t d", p=P), qnb[:]
            )
            # zero fills for this slot (execute during compute)
            for qt in range(T, NT):
                nc.gpsimd.dma_start(sc_d[b, qt * P : (qt + 1) * P, :], zt[:])
                nc.gpsimd.dma_start(
                    out_d[b, qt * P : (qt + 1) * P, D : 2 * D], zt[:, 0:D]
                )

            # mm1: sigT[k, q] = sigmoid(inv_c[k] * (cT.T @ qT) + mask)
            for qc in range(NQC):
                wq = min(512, W - qc * 512)
                nqb = wq // P
                for kt in range(T):
                    acc = ps1.tile([P, 512], F32, tag="acc")
                    for dch in range(ND):
                        nc.tensor.matmul(
                            acc[:, 0:wq],
                            cT[:, dch, kt * P : (kt + 1) * P],
                            qT[:, dch, qc * 512 : qc * 512 + wq],
                            start=(dch == 0),
                            stop=(dch == ND - 1),
                        )
                    nc.scalar.activation(
                        sg[:, qc * 4 : qc * 4 + nqb, kt, :], acc[:, 0:wq],
                        AF.Sigmoid, bias=mk[:, kt : kt + 1],
                        scale=inv[:, ccol(kt) : ccol(kt) + 1],
                    )

            slots[b] = dict(T=T, W=W, mk=mk, cbt=cbt, sg=sg)

        def phase2(b):
            """per q-block: xbar score transpose, den, w, attended, writes."""
            st = slots.pop(b)
            T, W, mk, cbt, sg = st["T"], st["W"], st["mk"], st["cbt"], st["sg"]
            aob = qbp.tile([P, T, D], BF16, tag="aob")
            for qb in range(T):
                so = opool.tile([P, T, P], BF16, tag="so")
                if W < S:
                    nc.gpsimd.dma_start(
                        sc_d[b, qb * P : (qb + 1) * P, W:S], zt[:, 0 : S - W]
                    )
                NKG = (T + 3) // 4
                dps = []
                for kg in range(NKG):
                    G = min(4, T - kg * 4)
                    pt = pst.tile([P, ND, P], BF16, tag="pt")
                    for j in range(G):
                        kt = kg * 4 + j
                        nc.tensor.transpose(pt[:, j], sg[:, qb, kt, :], idb[:])
                    # evict unscaled sigT^T; denominator rides along in
                    # the activation/tensor-scalar accumulator
                    dp = mpool.tile([P, 1], F32, tag=f"dp{kg}")
                    dps.append(dp)
                    if kg % 2 == 0:
                        nc.scalar.activation(
                            so[:, kg * 4 : kg * 4 + G, :], pt[:, 0:G],
                            AF.Copy, accum_out=dp[:],
                        )
                    else:
                        nc.vector.tensor_scalar(
                            so[:, kg * 4 : kg * 4 + G, :], pt[:, 0:G],
                            1.0, None, op0=ALU.mult, op1=ALU.add,
                            accum_out=dp[:],
                        )

                att = ps2.tile([P, 512], F32, tag="att")
                for kt in range(T):
                    nc.tensor.matmul(
                        att[:], sg[:, qb, kt, :], cbt[:, kt],
                        start=(kt == 0), stop=(kt == T - 1),
                    )

                # w = qmask / max(den, 1)
                den = mpool.tile([P, 1], F32, tag="den")
                w = mpool.tile([P, 1], F32, tag="w")
                if NKG == 2:
                    nc.vector.tensor_add(den[:], dps[0][:], dps[1][:])
                else:
                    nc.vector.tensor_copy(den[:], dps[0][:])
                nc.vector.tensor_scalar_max(den[:], den[:], 1.0)
                nc.vector.reciprocal(w[:], den[:])
                nc.vector.tensor_mul(w[:], w[:], mk[:, NT + qb : NT + qb + 1])

                # scale scores in place, write out
                nc.vector.tensor_scalar_mul(so[:], so[:], w[:])
                nc.gpsimd.dma_start(sc_d[b, qb * P : (qb + 1) * P, 0:W], so[:])

                nc.vector.tensor_scalar_mul(aob[:, qb], att[:], w[:])

            nc.scalar.dma_start(
                out_d[b, 0:W, D : 2 * D].rearrange("(t p) d -> p t d", p=P),
                aob[:],
            )

        # software-pipelined emission: slot b+1's front half is queued
        # before slot b's back half.
        phase1(0)
        for b in range(BPC):
            if b + 1 < BPC:
                phase1(b + 1)
            phase2(b)


_NC_CACHE = {}


def _get_nc(ts):
    key = ("nc", ts)
    if key not in _NC_CACHE:
        _NC_CACHE[key] = build_kernel(ts)
    return _NC_CACHE[key]


def plan(length):
    """Sort batches by tile count desc, deal round-robin to cores.

    Returns (ts, order): ts[j] = baked tile count for slot j; order[j*NCORES+c]
    = batch index placed in slot j of core c.
    """
    length = np.asarray(length).astype(np.int64)
    T = np.ceil(length / P).astype(np.int64)
    order = np.argsort(-T, kind="stable")
    ts = tuple(int(T[order[j * NCORES]]) for j in range(BPC))
    return ts, order


def prep_inputs(context, query, length):
    context = np.ascontiguousarray(np.asarray(context, dtype=np.float32))
    query = np.ascontiguousarray(np.asarray(query, dtype=np.float32))
    length = np.asarray(length).astype(np.int64)
    ts, order = plan(length)

    iot = np.arange(S)
    keymask = iot[None, :] < length[:, None]                      # [B, S]
    kbH = np.where(keymask, np.float32(0.0), NEG).astype(np.float32)
    kbH = kbH.reshape(B, NT, P).transpose(0, 2, 1)
    qmH = keymask.astype(np.float32).reshape(B, NT, P).transpose(0, 2, 1)
    mkH = np.ascontiguousarray(np.concatenate([kbH, qmH], axis=2))
    idb = np.eye(P, dtype=ml_dtypes.bfloat16)

    in_maps = []
    for c in range(NCORES):
        bidx = [int(order[j * NCORES + c]) for j in range(BPC)]
        in_maps.append(
            {
                "query": np.ascontiguousarray(query[bidx]),
                "context": np.ascontiguousarray(context[bidx]),
                "masks": np.ascontiguousarray(mkH[bidx]),
                "identity": idb,
            }
        )
    return ts, order, in_maps


def kernel(context, query, length):
    ts, order, in_maps = prep_inputs(context, query, length)
    nc = _get_nc(ts)
    res = run_bass_kernel_spmd(nc, in_maps, list(range(NCORES)))
    _NC_CACHE["last_result"] = res

    out = np.empty((B, S, 2 * D), np.float32)
    scores = np.empty((B, S, S), np.float32)
    for c in range(NCORES):
        ro = np.asarray(res.results[c]["out"]).astype(np.float32)
        rs = np.asarray(res.results[c]["scores"]).astype(np.float32)
        for j in range(BPC):
            bi = int(order[j * NCORES + c])
            out[bi] = ro[j]
            scores[bi] = rs[j]
    return out, scores


# revision 19
# speedup vs baseline: 1.7015x; 1.0569x over previous
"""Trainium2 Bass kernel for BiLinearSigmoidAttention (length-sparse, bf16).

Reference math (per batch b, with L = length[b]):
    qn = l2norm(query), cn = l2norm(context)
    raw[q,k] = qn[q] . cn[k]            (masked: k >= L -> -1e30)
    sig = sigmoid(raw)
    den[q] = max(sum_k sig[q,k], 1)
    scores[q,k] = sig[q,k] / den[q]     (rows q >= L zeroed)
    att[q,:] = sum_k scores[q,k] * context[k,:]
    out = concat([qn, att], -1)
returns (out [B,S,2D], scores [B,S,S])

Key structure (8 NeuronCores, data parallel over B=32 -> 4 slots per core):
  - sigmoid(-1e30) == 0, so only the first T_b = ceil(L_b/128) row/col
    tile-blocks of the [S,S] score matrix are nonzero. Batches are sorted
    by T descending and dealt round-robin to cores; slot j of every core
    runs with the baked tile count ts[j] = max T in that deal group.
    Zero regions are DMA'd from a zeroed SBUF tile during compute.
  - all matmuls and PE transposes run in bf16 (tolerance is 2e-2);
    outputs are written bf16 and upcast to fp32 on the host.
  - emission is software-pipelined: slot b+1's input DMAs and front-half
    compute are emitted before slot b's per-q-block phase, so input
    streaming and PE work never starve at slot boundaries.
  - qT/cT transposes run on the PE (cheap in bf16); the per-q-block score
    transpose uses one DMA-xbar transfer (dma_start_transpose, extra-major
    row mapping) per block: sg is stored [k', qb, kt, q_local] so the
    transfer yields the scores row block directly, PE runs only matmuls
    in the back half.
  - scalar activation functions are grouped (Square/Sqrt, then Sigmoid,
    then table-free Copies): Sigmoid <-> Square/Sqrt transitions cost a
    ~1.3us activation-table reload.
  - mm1 computes sigT [k_part, q_free]; the length mask is a per-partition
    bias and the context l2-norm a per-partition scale fused into the
    sigmoid activation; ps1 holds 4 PSUM banks so matmuls run ahead of
    the norm-gated sigmoid evictions.
  - DMA dispatch spread over three queues: inputs + score xbar on sync,
    ao on scalar (HWDGE), qn/score rows/zero fills on gpsimd (SWDGE).
"""

import numpy as np
import ml_dtypes

import concourse.bacc as bacc
import concourse.mybir as mybir
import concourse.tile as tile
from concourse.bass_utils import run_bass_kernel_spmd

B, S, D = 32, 1024, 512
NCORES = 8
BPC = B // NCORES          # batch slots per core
P = 128                    # partitions
NT = S // P                # 8 s-tiles
ND = D // P                # 4 d-chunks
NEG = np.float32(-1e30)

F32 = mybir.dt.float32
BF16 = mybir.dt.bfloat16
AF = mybir.ActivationFunctionType
ALU = mybir.AluOpType
AX = mybir.AxisListType


def build_kernel(ts):
    """ts: per-slot baked tile counts (len BPC, descending, each 1..NT)."""
    nc = bacc.Bacc("TRN2", target_bir_lowering=False, debug=False)

    q_d = nc.dram_tensor("query", [BPC, S, D], F32, kind="ExternalInput")
    c_d = nc.dram_tensor("context", [BPC, S, D], F32, kind="ExternalInput")
    # masks[b, p, kt]      = 0 if kt*P+p < L else -1e30   (cols 0..NT)
    # masks[b, p, NT + qb] = 1 if qb*P+p < L else 0       (cols NT..2NT)
    mk_d = nc.dram_tensor("masks", [BPC, P, 2 * NT], F32, kind="ExternalInput")
    id_d = nc.dram_tensor("identity", [P, P], BF16, kind="ExternalInput")
    out_d = nc.dram_tensor("out", [BPC, S, 2 * D], BF16, kind="ExternalOutput")
    sc_d = nc.dram_tensor("scores", [BPC, S, S], BF16, kind="ExternalOutput")

    with tile.TileContext(nc) as tc:
        _body(tc, ts, q_d, c_d, mk_d, id_d, out_d, sc_d)
    nc.compile()
    return nc


def _body(tc, ts, q_d, c_d, mk_d, id_d, out_d, sc_d):
    nc = tc.nc
    from contextlib import ExitStack

    ctx = ExitStack()
    with ctx:
        const = ctx.enter_context(tc.tile_pool(name="const", bufs=1))
        qpool = ctx.enter_context(tc.tile_pool(name="q", bufs=2))
        cpool = ctx.enter_context(tc.tile_pool(name="c", bufs=2))
        qbp = ctx.enter_context(tc.tile_pool(name="qb", bufs=2))
        cbp = ctx.enter_context(tc.tile_pool(name="cb", bufs=2))
        tp = ctx.enter_context(tc.tile_pool(name="t", bufs=2))
        sgp = ctx.enter_context(tc.tile_pool(name="sg", bufs=2))
        mpool = ctx.enter_context(tc.tile_pool(name="m", bufs=2))
        spool = ctx.enter_context(tc.tile_pool(name="s", bufs=3))
        opool = ctx.enter_context(tc.tile_pool(name="o", bufs=3))
        ps1 = ctx.enter_context(tc.tile_pool(name="ps1", bufs=4, space="PSUM"))
        pst = ctx.enter_context(tc.tile_pool(name="pst", bufs=2, space="PSUM"))
        ps2 = ctx.enter_context(tc.tile_pool(name="ps2", bufs=2, space="PSUM"))

        idb = const.tile([P, P], BF16, tag="idb")
        nc.sync.dma_start(idb[:], id_d[:])
        zt = const.tile([P, S], BF16, tag="zt")
        nc.gpsimd.memset(zt[:], 0.0)

        slots = {}

        def inputs(b):
            """input DMAs only (sync queue), prefetched one slot ahead."""
            T = ts[b]
            W = T * P
            qt_t = qpool.tile([P, NT, D], F32, tag="qt")
            ct_t = cpool.tile([P, NT, D], F32, tag="ct")
            mk = mpool.tile([P, 2 * NT], F32, tag="mk")
            nc.sync.dma_start(ct_t[:, 0], c_d[b, 0:P, :])
            nc.sync.dma_start(
                qt_t[:, 0:4], q_d[b, 0:512, :].rearrange("(t p) d -> p t d", p=P)
            )
            if T > 1:
                nc.sync.dma_start(
                    ct_t[:, 1:T],
                    c_d[b, P:W, :].rearrange("(t p) d -> p t d", p=P),
                )
            nc.sync.dma_start(
                qt_t[:, 4:NT],
                q_d[b, 512:S, :].rearrange("(t p) d -> p t d", p=P),
            )
            nc.sync.dma_start(mk[:], mk_d[b])
            slots[b] = dict(qt_t=qt_t, ct_t=ct_t, mk=mk)

        def compute1(b):
            """norms, qn/cbt, qT/cT transposes, mm1+sigmoid."""
            T = ts[b]
            W = T * P
            NQC = (W + 511) // 512

            def ccol(kt):
                return 0 if kt == 0 else 4 + kt

            def qcol(t):
                return 1 + t if t < 4 else T + t

            NC_ = NT + T
            qt_t = slots[b]["qt_t"]
            ct_t = slots[b]["ct_t"]
            mk = slots[b]["mk"]

            ssq = mpool.tile([P, NT + NT], F32, tag="ssq")
            nrm = mpool.tile([P, NT + NT], F32, tag="nrm")
            inv = mpool.tile([P, NT + NT], F32, tag="inv")
            qnb = qbp.tile([P, NT, D], BF16, tag="qnb")
            cbt = cbp.tile([P, NT, D], BF16, tag="cbt")
            qT = tp.tile([P, ND, W], BF16, tag="qT")
            cT = tp.tile([P, ND, W], BF16, tag="cT")
            # sg[k', qb, kt, q_local]: one xbar transfer per qb gives the
            # scores row block [q, kt*P + k'] directly.
            sg = sgp.tile([P, T, T, P], BF16, tag="sg")

            def square(col, src):
                scr2 = spool.tile([P, D], BF16, tag="scr2")
                nc.scalar.activation(
                    scr2[:], src, AF.Square, accum_out=ssq[:, col : col + 1]
                )

            def transpose_tile(src, dst_T, t, evict_vec):
                pq = pst.tile([P, ND, P], BF16, tag="pt")
                for dch in range(ND):
                    nc.tensor.transpose(
                        pq[:, dch], src[:, dch * P : (dch + 1) * P], idb[:]
                    )
                if evict_vec:
                    nc.vector.tensor_copy(dst_T[:, :, t * P : (t + 1) * P], pq[:])
                else:
                    nc.scalar.copy(dst_T[:, :, t * P : (t + 1) * P], pq[:])

            # first half: c0 + q0..q3 norms (scalar Square/Sqrt block)
            square(ccol(0), ct_t[:, 0])
            for t in range(4):
                square(qcol(t), qt_t[:, t])
            nc.scalar.activation(nrm[:, 0:5], ssq[:, 0:5], AF.Sqrt)
            nc.vector.reciprocal(inv[:, 0:5], nrm[:, 0:5])

            nc.vector.tensor_copy(cbt[:, 0], ct_t[:, 0])
            for t in range(4):
                nc.vector.tensor_scalar_mul(
                    qnb[:, t], qt_t[:, t], inv[:, qcol(t) : qcol(t) + 1]
                )
            transpose_tile(cbt[:, 0], cT, 0, evict_vec=True)
            for t in range(min(4, T)):
                transpose_tile(qnb[:, t], qT, t, evict_vec=False)

            # second half: c1..cT-1 + q4..q7 norms
            for kt in range(1, T):
                square(ccol(kt), ct_t[:, kt])
            for t in range(4, NT):
                square(qcol(t), qt_t[:, t])
            if NC_ > 5:
                nc.scalar.activation(nrm[:, 5:NC_], ssq[:, 5:NC_], AF.Sqrt)
                nc.vector.reciprocal(inv[:, 5:NC_], nrm[:, 5:NC_])

            for kt in range(1, T):
                nc.vector.tensor_copy(cbt[:, kt], ct_t[:, kt])
                transpose_tile(cbt[:, kt], cT, kt, evict_vec=True)
            for t in range(4, NT):
                nc.vector.tensor_scalar_mul(
                    qnb[:, t], qt_t[:, t], inv[:, qcol(t) : qcol(t) + 1]
                )
                if t < T:
                    transpose_tile(qnb[:, t], qT, t, evict_vec=False)

            nc.gpsimd.dma_start(
                out_d[b, :, 0:D].rearrange("(t p) d -> p t d", p=P), qnb[:]
            )
            # zero fills for this slot (execute during compute)
            for qt in range(T, NT):
                nc.gpsimd.dma_start(sc_d[b, qt * P : (qt + 1) * P, :], zt[:])
                nc.gpsimd.dma_start(
                    out_d[b, qt * P : (qt + 1) * P, D : 2 * D], zt[:, 0:D]
                )

            # mm1: sigT[k, q] = sigmoid(inv_c[k] * (cT.T @ qT) + mask)
            for qc in range(NQC):
                wq = min(512, W - qc * 512)
                nqb = wq // P
                for kt in range(T):
                    acc = ps1.tile([P, 512], F32, tag="acc")
                    for dch in range(ND):
                        nc.tensor.matmul(
                            acc[:, 0:wq],
                            cT[:, dch, kt * P : (kt + 1) * P],
                            qT[:, dch, qc * 512 : qc * 512 + wq],
                            start=(dch == 0),
                            stop=(dch == ND - 1),
                        )
                    nc.scalar.activation(
                        sg[:, qc * 4 : qc * 4 + nqb, kt, :], acc[:, 0:wq],
                        AF.Sigmoid, bias=mk[:, kt : kt + 1],
                        scale=inv[:, ccol(kt) : ccol(kt) + 1],
                    )

            slots[b].update(T=T, W=W, cbt=cbt, sg=sg)

        def phase2(b):
            """per q-block: xbar score transpose, den, w, attended, writes."""
            st = slots.pop(b)
            T, W, mk, cbt, sg = st["T"], st["W"], st["mk"], st["cbt"], st["sg"]
            aob = qbp.tile([P, T, D], BF16, tag="aob")
            for qb in range(T):
                so = opool.tile([P, T, P], BF16, tag="so")
                if W < S:
                    nc.gpsimd.dma_start(
                        sc_d[b, qb * P : (qb + 1) * P, W:S], zt[:, 0 : S - W]
                    )
                NKG = (T + 3) // 4
                dps = []
                for kg in range(NKG):
                    G = min(4, T - kg * 4)
                    pt = pst.tile([P, ND, P], BF16, tag="pt")
                    for j in range(G):
                        kt = kg * 4 + j
                        nc.tensor.transpose(pt[:, j], sg[:, qb, kt, :], idb[:])
                    # evict unscaled sigT^T; denominator rides along in
                    # the activation/tensor-scalar accumulator
                    dp = mpool.tile([P, 1], F32, tag=f"dp{kg}")
                    dps.append(dp)
                    if kg % 2 == 0:
                        nc.scalar.activation(
                            so[:, kg * 4 : kg * 4 + G, :], pt[:, 0:G],
                            AF.Copy, accum_out=dp[:],
                        )
                    else:
                        nc.vector.tensor_scalar(
                            so[:, kg * 4 : kg * 4 + G, :], pt[:, 0:G],
                            1.0, None, op0=ALU.mult, op1=ALU.add,
                            accum_out=dp[:],
                        )

                att = ps2.tile([P, 512], F32, tag="att")
                for kt in range(T):
                    nc.tensor.matmul(
                        att[:], sg[:, qb, kt, :], cbt[:, kt],
                        start=(kt == 0), stop=(kt == T - 1),
                    )

                # w = qmask / max(den, 1)
                den = mpool.tile([P, 1], F32, tag="den")
                w = mpool.tile([P, 1], F32, tag="w")
                if NKG == 2:
                    nc.vector.tensor_add(den[:], dps[0][:], dps[1][:])
                else:
                    nc.vector.tensor_copy(den[:], dps[0][:])
                nc.vector.tensor_scalar_max(den[:], den[:], 1.0)
                nc.vector.reciprocal(w[:], den[:])
                nc.vector.tensor_mul(w[:], w[:], mk[:, NT + qb : NT + qb + 1])

                # scale scores in place, write out
                nc.vector.tensor_scalar_mul(so[:], so[:], w[:])
                nc.gpsimd.dma_start(sc_d[b, qb * P : (qb + 1) * P, 0:W], so[:])

                nc.vector.tensor_scalar_mul(aob[:, qb], att[:], w[:])

            nc.scalar.dma_start(
                out_d[b, 0:W, D : 2 * D].rearrange("(t p) d -> p t d", p=P),
                aob[:],
            )

        # sequential compute emission with input DMAs prefetched one slot
        # ahead (sync queue carries only input dispatches, so this only
        # reorders transfers, not compute).
        inputs(0)
        compute1(0)
        for b in range(BPC):
            if b + 1 < BPC:
                inputs(b + 1)
            phase2(b)
            if b + 1 < BPC:
                compute1(b + 1)


_NC_CACHE = {}


def _get_nc(ts):
    key = ("nc", ts)
    if key not in _NC_CACHE:
        _NC_CACHE[key] = build_kernel(ts)
    return _NC_CACHE[key]


def plan(length):
    """Sort batches by tile count desc, deal round-robin to cores.

    Returns (ts, order): ts[j] = baked tile count for slot j; order[j*NCORES+c]
    = batch index placed in slot j of core c.
    """
    length = np.asarray(length).astype(np.int64)
    T = np.ceil(length / P).astype(np.int64)
    order = np.argsort(-T, kind="stable")
    ts = tuple(int(T[order[j * NCORES]]) for j in range(BPC))
    return ts, order


def prep_inputs(context, query, length):
    context = np.ascontiguousarray(np.asarray(context, dtype=np.float32))
    query = np.ascontiguousarray(np.asarray(query, dtype=np.float32))
    length = np.asarray(length).astype(np.int64)
    ts, order = plan(length)

    iot = np.arange(S)
    keymask = iot[None, :] < length[:, None]                      # [B, S]
    kbH = np.where(keymask, np.float32(0.0), NEG).astype(np.float32)
    kbH = kbH.reshape(B, NT, P).transpose(0, 2, 1)
    qmH = keymask.astype(np.float32).reshape(B, NT, P).transpose(0, 2, 1)
    mkH = np.ascontiguousarray(np.concatenate([kbH, qmH], axis=2))
    idb = np.eye(P, dtype=ml_dtypes.bfloat16)

    in_maps = []
    for c in range(NCORES):
        bidx = [int(order[j * NCORES + c]) for j in range(BPC)]
        in_maps.append(
            {
                "query": np.ascontiguousarray(query[bidx]),
                "context": np.ascontiguousarray(context[bidx]),
                "masks": np.ascontiguousarray(mkH[bidx]),
                "identity": idb,
            }
        )
    return ts, order, in_maps


def kernel(context, query, length):
    ts, order, in_maps = prep_inputs(context, query, length)
    nc = _get_nc(ts)
    res = run_bass_kernel_spmd(nc, in_maps, list(range(NCORES)))
    _NC_CACHE["last_result"] = res

    out = np.empty((B, S, 2 * D), np.float32)
    scores = np.empty((B, S, S), np.float32)
    for c in range(NCORES):
        ro = np.asarray(res.results[c]["out"]).astype(np.float32)
        rs = np.asarray(res.results[c]["scores"]).astype(np.float32)
        for j in range(BPC):
            bi = int(order[j * NCORES + c])
            out[bi] = ro[j]
            scores[bi] = rs[j]
    return out, scores


# revision 20
# speedup vs baseline: 1.8262x; 1.0733x over previous
"""Trainium2 Bass kernel for BiLinearSigmoidAttention (length-sparse, bf16).

Reference math (per batch b, with L = length[b]):
    qn = l2norm(query), cn = l2norm(context)
    raw[q,k] = qn[q] . cn[k]            (masked: k >= L -> -1e30)
    sig = sigmoid(raw)
    den[q] = max(sum_k sig[q,k], 1)
    scores[q,k] = sig[q,k] / den[q]     (rows q >= L zeroed)
    att[q,:] = sum_k scores[q,k] * context[k,:]
    out = concat([qn, att], -1)
returns (out [B,S,2D], scores [B,S,S])

Key structure (8 NeuronCores, data parallel over B=32 -> 4 slots per core):
  - sigmoid(-1e30) == 0, so only the first T_b = ceil(L_b/128) row/col
    tile-blocks of the [S,S] score matrix are nonzero. Batches are sorted
    by T descending and dealt round-robin to cores; slot j of every core
    runs with the baked tile count ts[j] = max T in that deal group.
    Zero regions are DMA'd from a zeroed SBUF tile during compute.
  - all matmuls and PE transposes run in bf16 (tolerance is 2e-2);
    outputs are written bf16 and upcast to fp32 on the host.
  - emission is software-pipelined: slot b+1's input DMAs and front-half
    compute are emitted before slot b's per-q-block phase, so input
    streaming and PE work never starve at slot boundaries.
  - qT/cT transposes run on the PE (cheap in bf16); the per-q-block score
    transpose uses one DMA-xbar transfer (dma_start_transpose, extra-major
    row mapping) per block: sg is stored [k', qb, kt, q_local] so the
    transfer yields the scores row block directly, PE runs only matmuls
    in the back half.
  - scalar activation functions are grouped (Square/Sqrt, then Sigmoid,
    then table-free Copies): Sigmoid <-> Square/Sqrt transitions cost a
    ~1.3us activation-table reload.
  - mm1 computes sigT [k_part, q_free]; the length mask is a per-partition
    bias and the context l2-norm a per-partition scale fused into the
    sigmoid activation; ps1 holds 4 PSUM banks so matmuls run ahead of
    the norm-gated sigmoid evictions.
  - DMA dispatch spread over three queues: inputs + score xbar on sync,
    ao on scalar (HWDGE), qn/score rows/zero fills on gpsimd (SWDGE).
"""

import numpy as np
import ml_dtypes

import concourse.bacc as bacc
import concourse.mybir as mybir
import concourse.tile as tile
from concourse.bass_utils import run_bass_kernel_spmd

B, S, D = 32, 1024, 512
NCORES = 8
BPC = B // NCORES          # batch slots per core
P = 128                    # partitions
NT = S // P                # 8 s-tiles
ND = D // P                # 4 d-chunks
NEG = np.float32(-1e30)

F32 = mybir.dt.float32
BF16 = mybir.dt.bfloat16
AF = mybir.ActivationFunctionType
ALU = mybir.AluOpType
AX = mybir.AxisListType


def build_kernel(ts):
    """ts: per-slot baked tile counts (len BPC, descending, each 1..NT)."""
    nc = bacc.Bacc("TRN2", target_bir_lowering=False, debug=False)

    q_d = nc.dram_tensor("query", [BPC, S, D], F32, kind="ExternalInput")
    c_d = nc.dram_tensor("context", [BPC, S, D], F32, kind="ExternalInput")
    # masks[b, p, kt]      = 0 if kt*P+p < L else -1e30   (cols 0..NT)
    # masks[b, p, NT + qb] = 1 if qb*P+p < L else 0       (cols NT..2NT)
    mk_d = nc.dram_tensor("masks", [BPC, P, 2 * NT], F32, kind="ExternalInput")
    id_d = nc.dram_tensor("identity", [P, P], BF16, kind="ExternalInput")
    out_d = nc.dram_tensor("out", [BPC, S, 2 * D], BF16, kind="ExternalOutput")
    sc_d = nc.dram_tensor("scores", [BPC, S, S], BF16, kind="ExternalOutput")

    with tile.TileContext(nc) as tc:
        _body(tc, ts, q_d, c_d, mk_d, id_d, out_d, sc_d)
    nc.compile()
    return nc


def _body(tc, ts, q_d, c_d, mk_d, id_d, out_d, sc_d):
    nc = tc.nc
    from contextlib import ExitStack

    ctx = ExitStack()
    with ctx:
        const = ctx.enter_context(tc.tile_pool(name="const", bufs=1))
        qpool = ctx.enter_context(tc.tile_pool(name="q", bufs=2))
        cpool = ctx.enter_context(tc.tile_pool(name="c", bufs=2))
        qbp = ctx.enter_context(tc.tile_pool(name="qb", bufs=2))
        cbp = ctx.enter_context(tc.tile_pool(name="cb", bufs=2))
        tp = ctx.enter_context(tc.tile_pool(name="t", bufs=2))
        sgp = ctx.enter_context(tc.tile_pool(name="sg", bufs=2))
        mpool = ctx.enter_context(tc.tile_pool(name="m", bufs=2))
        spool = ctx.enter_context(tc.tile_pool(name="s", bufs=3))
        opool = ctx.enter_context(tc.tile_pool(name="o", bufs=3))
        ps1 = ctx.enter_context(tc.tile_pool(name="ps1", bufs=4, space="PSUM"))
        pst = ctx.enter_context(tc.tile_pool(name="pst", bufs=2, space="PSUM"))
        ps2 = ctx.enter_context(tc.tile_pool(name="ps2", bufs=2, space="PSUM"))

        idb = const.tile([P, P], BF16, tag="idb")
        nc.sync.dma_start(idb[:], id_d[:])
        zt = const.tile([P, S], BF16, tag="zt")
        nc.gpsimd.memset(zt[:], 0.0)

        slots = {}

        def inputs(b):
            """input DMAs only (sync queue), prefetched one slot ahead."""
            T = ts[b]
            W = T * P
            qt_t = qpool.tile([P, NT, D], F32, tag="qt")
            ct_t = cpool.tile([P, NT, D], F32, tag="ct")
            mk = mpool.tile([P, 2 * NT], F32, tag="mk")
            nc.sync.dma_start(ct_t[:, 0], c_d[b, 0:P, :])
            nc.sync.dma_start(
                qt_t[:, 0:4], q_d[b, 0:512, :].rearrange("(t p) d -> p t d", p=P)
            )
            if T > 1:
                nc.sync.dma_start(
                    ct_t[:, 1:T],
                    c_d[b, P:W, :].rearrange("(t p) d -> p t d", p=P),
                )
            nc.sync.dma_start(
                qt_t[:, 4:NT],
                q_d[b, 512:S, :].rearrange("(t p) d -> p t d", p=P),
            )
            nc.sync.dma_start(mk[:], mk_d[b])
            slots[b] = dict(qt_t=qt_t, ct_t=ct_t, mk=mk)

        def compute1(b):
            """norms, qn/cbt, qT/cT transposes, mm1+sigmoid."""
            T = ts[b]
            W = T * P
            NQC = (W + 511) // 512

            def ccol(kt):
                return 0 if kt == 0 else 4 + kt

            def qcol(t):
                return 1 + t if t < 4 else T + t

            NC_ = NT + T
            qt_t = slots[b]["qt_t"]
            ct_t = slots[b]["ct_t"]
            mk = slots[b]["mk"]

            ssq = mpool.tile([P, NT + NT], F32, tag="ssq")
            nrm = mpool.tile([P, NT + NT], F32, tag="nrm")
            inv = mpool.tile([P, NT + NT], F32, tag="inv")
            qnb = qbp.tile([P, NT, D], BF16, tag="qnb")
            cbt = cbp.tile([P, NT, D], BF16, tag="cbt")
            qT = tp.tile([P, ND, W], BF16, tag="qT")
            cT = tp.tile([P, ND, W], BF16, tag="cT")
            # sg[k', qb, kt, q_local]: one xbar transfer per qb gives the
            # scores row block [q, kt*P + k'] directly.
            sg = sgp.tile([P, T, T, P], BF16, tag="sg")

            def square(col, src):
                scr2 = spool.tile([P, D], BF16, tag="scr2")
                nc.scalar.activation(
                    scr2[:], src, AF.Square, accum_out=ssq[:, col : col + 1]
                )

            def transpose_tile(src, dst_T, t, evict_vec):
                pq = pst.tile([P, ND, P], BF16, tag="pt")
                for dch in range(ND):
                    nc.tensor.transpose(
                        pq[:, dch], src[:, dch * P : (dch + 1) * P], idb[:]
                    )
                nc.vector.tensor_copy(dst_T[:, :, t * P : (t + 1) * P], pq[:])

            # first half: c0 + q0..q3 norms (scalar Square/Sqrt block)
            square(ccol(0), ct_t[:, 0])
            for t in range(4):
                square(qcol(t), qt_t[:, t])
            nc.scalar.activation(nrm[:, 0:5], ssq[:, 0:5], AF.Sqrt)
            nc.vector.reciprocal(inv[:, 0:5], nrm[:, 0:5])

            nc.vector.tensor_copy(cbt[:, 0], ct_t[:, 0])
            for t in range(4):
                nc.vector.tensor_scalar_mul(
                    qnb[:, t], qt_t[:, t], inv[:, qcol(t) : qcol(t) + 1]
                )
            transpose_tile(cbt[:, 0], cT, 0, evict_vec=True)
            for t in range(min(4, T)):
                transpose_tile(qnb[:, t], qT, t, evict_vec=False)

            # second half: c1..cT-1 + q4..q7 norms
            for kt in range(1, T):
                square(ccol(kt), ct_t[:, kt])
            for t in range(4, NT):
                square(qcol(t), qt_t[:, t])
            if NC_ > 5:
                nc.scalar.activation(nrm[:, 5:NC_], ssq[:, 5:NC_], AF.Sqrt)
                nc.vector.reciprocal(inv[:, 5:NC_], nrm[:, 5:NC_])

            for kt in range(1, T):
                nc.vector.tensor_copy(cbt[:, kt], ct_t[:, kt])
                transpose_tile(cbt[:, kt], cT, kt, evict_vec=True)
            for t in range(4, NT):
                nc.vector.tensor_scalar_mul(
                    qnb[:, t], qt_t[:, t], inv[:, qcol(t) : qcol(t) + 1]
                )
                if t < T:
                    transpose_tile(qnb[:, t], qT, t, evict_vec=False)

            nc.gpsimd.dma_start(
                out_d[b, :, 0:D].rearrange("(t p) d -> p t d", p=P), qnb[:]
            )
            # zero fills for this slot (execute during compute)
            for qt in range(T, NT):
                nc.gpsimd.dma_start(sc_d[b, qt * P : (qt + 1) * P, :], zt[:])
                nc.gpsimd.dma_start(
                    out_d[b, qt * P : (qt + 1) * P, D : 2 * D], zt[:, 0:D]
                )

            # mm1: sigT[k, q] = sigmoid(inv_c[k] * (cT.T @ qT) + mask)
            for qc in range(NQC):
                wq = min(512, W - qc * 512)
                nqb = wq // P
                for kt in range(T):
                    acc = ps1.tile([P, 512], F32, tag="acc")
                    for dch in range(ND):
                        nc.tensor.matmul(
                            acc[:, 0:wq],
                            cT[:, dch, kt * P : (kt + 1) * P],
                            qT[:, dch, qc * 512 : qc * 512 + wq],
                            start=(dch == 0),
                            stop=(dch == ND - 1),
                        )
                    nc.scalar.activation(
                        sg[:, qc * 4 : qc * 4 + nqb, kt, :], acc[:, 0:wq],
                        AF.Sigmoid, bias=mk[:, kt : kt + 1],
                        scale=inv[:, ccol(kt) : ccol(kt) + 1],
                    )

            slots[b].update(T=T, W=W, cbt=cbt, sg=sg)

        def phase2(b):
            """per q-block: xbar score transpose, den, w, attended, writes."""
            st = slots.pop(b)
            T, W, mk, cbt, sg = st["T"], st["W"], st["mk"], st["cbt"], st["sg"]
            aob = qbp.tile([P, T, D], BF16, tag="aob")
            for qb in range(T):
                so = opool.tile([P, T, P], BF16, tag="so")
                if W < S:
                    nc.gpsimd.dma_start(
                        sc_d[b, qb * P : (qb + 1) * P, W:S], zt[:, 0 : S - W]
                    )
                NKG = (T + 3) // 4
                dps = []
                for kg in range(NKG):
                    G = min(4, T - kg * 4)
                    pt = pst.tile([P, ND, P], BF16, tag="pt")
                    for j in range(G):
                        kt = kg * 4 + j
                        nc.tensor.transpose(pt[:, j], sg[:, qb, kt, :], idb[:])
                    # evict unscaled sigT^T; denominator rides along in
                    # the activation/tensor-scalar accumulator
                    dp = mpool.tile([P, 1], F32, tag=f"dp{kg}")
                    dps.append(dp)
                    nc.vector.tensor_scalar(
                        so[:, kg * 4 : kg * 4 + G, :], pt[:, 0:G],
                        1.0, None, op0=ALU.mult, op1=ALU.add,
                        accum_out=dp[:],
                    )

                att = ps2.tile([P, 512], F32, tag="att")
                for kt in range(T):
                    nc.tensor.matmul(
                        att[:], sg[:, qb, kt, :], cbt[:, kt],
                        start=(kt == 0), stop=(kt == T - 1),
                    )

                # w = qmask / max(den, 1)
                den = mpool.tile([P, 1], F32, tag="den")
                w = mpool.tile([P, 1], F32, tag="w")
                if NKG == 2:
                    nc.vector.tensor_add(den[:], dps[0][:], dps[1][:])
                else:
                    nc.vector.tensor_copy(den[:], dps[0][:])
                nc.vector.tensor_scalar_max(den[:], den[:], 1.0)
                nc.vector.reciprocal(w[:], den[:])
                nc.vector.tensor_mul(w[:], w[:], mk[:, NT + qb : NT + qb + 1])

                # scale scores in place, write out
                nc.vector.tensor_scalar_mul(so[:], so[:], w[:])
                nc.gpsimd.dma_start(sc_d[b, qb * P : (qb + 1) * P, 0:W], so[:])

                nc.vector.tensor_scalar_mul(aob[:, qb], att[:], w[:])

            nc.gpsimd.dma_start(
                out_d[b, 0:W, D : 2 * D].rearrange("(t p) d -> p t d", p=P),
                aob[:],
            )

        # sequential compute emission with input DMAs prefetched one slot
        # ahead (sync queue carries only input dispatches, so this only
        # reorders transfers, not compute).
        inputs(0)
        compute1(0)
        for b in range(BPC):
            if b + 1 < BPC:
                inputs(b + 1)
            phase2(b)
            if b + 1 < BPC:
                compute1(b + 1)


_NC_CACHE = {}


def _get_nc(ts):
    key = ("nc", ts)
    if key not in _NC_CACHE:
        _NC_CACHE[key] = build_kernel(ts)
    return _NC_CACHE[key]


def plan(length):
    """Sort batches by tile count desc, deal round-robin to cores.

    Returns (ts, order): ts[j] = baked tile count for slot j; order[j*NCORES+c]
    = batch index placed in slot j of core c.
    """
    length = np.asarray(length).astype(np.int64)
    T = np.ceil(length / P).astype(np.int64)
    order = np.argsort(-T, kind="stable")
    ts = tuple(int(T[order[j * NCORES]]) for j in range(BPC))
    return ts, order


def prep_inputs(context, query, length):
    context = np.ascontiguousarray(np.asarray(context, dtype=np.float32))
    query = np.ascontiguousarray(np.asarray(query, dtype=np.float32))
    length = np.asarray(length).astype(np.int64)
    ts, order = plan(length)

    iot = np.arange(S)
    keymask = iot[None, :] < length[:, None]                      # [B, S]
    kbH = np.where(keymask, np.float32(0.0), NEG).astype(np.float32)
    kbH = kbH.reshape(B, NT, P).transpose(0, 2, 1)
    qmH = keymask.astype(np.float32).reshape(B, NT, P).transpose(0, 2, 1)
    mkH = np.ascontiguousarray(np.concatenate([kbH, qmH], axis=2))
    idb = np.eye(P, dtype=ml_dtypes.bfloat16)

    in_maps = []
    for c in range(NCORES):
        bidx = [int(order[j * NCORES + c]) for j in range(BPC)]
        in_maps.append(
            {
                "query": np.ascontiguousarray(query[bidx]),
                "context": np.ascontiguousarray(context[bidx]),
                "masks": np.ascontiguousarray(mkH[bidx]),
                "identity": idb,
            }
        )
    return ts, order, in_maps


def kernel(context, query, length):
    ts, order, in_maps = prep_inputs(context, query, length)
    nc = _get_nc(ts)
    res = run_bass_kernel_spmd(nc, in_maps, list(range(NCORES)))
    _NC_CACHE["last_result"] = res

    out = np.empty((B, S, 2 * D), np.float32)
    scores = np.empty((B, S, S), np.float32)
    for c in range(NCORES):
        ro = np.asarray(res.results[c]["out"]).astype(np.float32)
        rs = np.asarray(res.results[c]["scores"]).astype(np.float32)
        for j in range(BPC):
            bi = int(order[j * NCORES + c])
            out[bi] = ro[j]
            scores[bi] = rs[j]
    return out, scores


# revision 21
# speedup vs baseline: 1.8738x; 1.0261x over previous
"""Trainium2 Bass kernel for BiLinearSigmoidAttention (length-sparse, bf16).

Reference math (per batch b, with L = length[b]):
    qn = l2norm(query), cn = l2norm(context)
    raw[q,k] = qn[q] . cn[k]            (masked: k >= L -> -1e30)
    sig = sigmoid(raw)
    den[q] = max(sum_k sig[q,k], 1)
    scores[q,k] = sig[q,k] / den[q]     (rows q >= L zeroed)
    att[q,:] = sum_k scores[q,k] * context[k,:]
    out = concat([qn, att], -1)
returns (out [B,S,2D], scores [B,S,S])

Key structure (8 NeuronCores, data parallel over B=32 -> 4 slots per core):
  - sigmoid(-1e30) == 0, so only the first T_b = ceil(L_b/128) row/col
    tile-blocks of the [S,S] score matrix are nonzero. Batches are sorted
    by T descending and dealt round-robin to cores; slot j of every core
    runs with the baked tile count ts[j] = max T in that deal group.
    Zero regions are DMA'd from a zeroed SBUF tile during compute.
  - all matmuls and PE transposes run in bf16 (tolerance is 2e-2);
    outputs are written bf16 and upcast to fp32 on the host.
  - emission is software-pipelined: slot b+1's input DMAs and front-half
    compute are emitted before slot b's per-q-block phase, so input
    streaming and PE work never starve at slot boundaries.
  - qT/cT transposes run on the PE (cheap in bf16); the per-q-block score
    transpose uses one DMA-xbar transfer (dma_start_transpose, extra-major
    row mapping) per block: sg is stored [k', qb, kt, q_local] so the
    transfer yields the scores row block directly, PE runs only matmuls
    in the back half.
  - scalar activation functions are grouped (Square/Sqrt, then Sigmoid,
    then table-free Copies): Sigmoid <-> Square/Sqrt transitions cost a
    ~1.3us activation-table reload.
  - mm1 computes sigT [k_part, q_free]; the length mask is a per-partition
    bias and the context l2-norm a per-partition scale fused into the
    sigmoid activation; ps1 holds 4 PSUM banks so matmuls run ahead of
    the norm-gated sigmoid evictions.
  - DMA dispatch spread over three queues: inputs + score xbar on sync,
    ao on scalar (HWDGE), qn/score rows/zero fills on gpsimd (SWDGE).
"""

import numpy as np
import ml_dtypes

import concourse.bacc as bacc
import concourse.mybir as mybir
import concourse.tile as tile
from concourse.bass_utils import run_bass_kernel_spmd

B, S, D = 32, 1024, 512
NCORES = 8
BPC = B // NCORES          # batch slots per core
P = 128                    # partitions
NT = S // P                # 8 s-tiles
ND = D // P                # 4 d-chunks
NEG = np.float32(-1e30)

F32 = mybir.dt.float32
BF16 = mybir.dt.bfloat16
AF = mybir.ActivationFunctionType
ALU = mybir.AluOpType
AX = mybir.AxisListType


def build_kernel(ts):
    """ts: per-slot baked tile counts (len BPC, descending, each 1..NT)."""
    nc = bacc.Bacc("TRN2", target_bir_lowering=False, debug=False)

    q_d = nc.dram_tensor("query", [BPC, S, D], F32, kind="ExternalInput")
    c_d = nc.dram_tensor("context", [BPC, S, D], F32, kind="ExternalInput")
    # masks[b, p, kt]      = 0 if kt*P+p < L else -1e30   (cols 0..NT)
    # masks[b, p, NT + qb] = 1 if qb*P+p < L else 0       (cols NT..2NT)
    mk_d = nc.dram_tensor("masks", [BPC, P, 2 * NT], F32, kind="ExternalInput")
    id_d = nc.dram_tensor("identity", [P, P], BF16, kind="ExternalInput")
    out_d = nc.dram_tensor("out", [BPC, S, 2 * D], BF16, kind="ExternalOutput")
    sc_d = nc.dram_tensor("scores", [BPC, S, S], BF16, kind="ExternalOutput")

    with tile.TileContext(nc) as tc:
        _body(tc, ts, q_d, c_d, mk_d, id_d, out_d, sc_d)
    nc.compile()
    return nc


def _body(tc, ts, q_d, c_d, mk_d, id_d, out_d, sc_d):
    nc = tc.nc
    from contextlib import ExitStack

    ctx = ExitStack()
    with ctx:
        const = ctx.enter_context(tc.tile_pool(name="const", bufs=1))
        qpool = ctx.enter_context(tc.tile_pool(name="q", bufs=2))
        cpool = ctx.enter_context(tc.tile_pool(name="c", bufs=2))
        qbp = ctx.enter_context(tc.tile_pool(name="qb", bufs=2))
        cbp = ctx.enter_context(tc.tile_pool(name="cb", bufs=2))
        tp = ctx.enter_context(tc.tile_pool(name="t", bufs=2))
        sgp = ctx.enter_context(tc.tile_pool(name="sg", bufs=2))
        mpool = ctx.enter_context(tc.tile_pool(name="m", bufs=2))
        spool = ctx.enter_context(tc.tile_pool(name="s", bufs=3))
        opool = ctx.enter_context(tc.tile_pool(name="o", bufs=3))
        ps1 = ctx.enter_context(tc.tile_pool(name="ps1", bufs=4, space="PSUM"))
        pst = ctx.enter_context(tc.tile_pool(name="pst", bufs=2, space="PSUM"))
        ps2 = ctx.enter_context(tc.tile_pool(name="ps2", bufs=2, space="PSUM"))

        idb = const.tile([P, P], BF16, tag="idb")
        nc.sync.dma_start(idb[:], id_d[:])
        zt = const.tile([P, S], BF16, tag="zt")
        nc.gpsimd.memset(zt[:], 0.0)

        slots = {}

        def inputs(b):
            """input DMAs only (sync queue), prefetched one slot ahead."""
            T = ts[b]
            W = T * P
            qt_t = qpool.tile([P, NT, D], F32, tag="qt")
            ct_t = cpool.tile([P, NT, D], F32, tag="ct")
            mk = mpool.tile([P, 2 * NT], F32, tag="mk")
            nc.scalar.dma_start(ct_t[:, 0], c_d[b, 0:P, :])
            nc.sync.dma_start(
                qt_t[:, 0:4], q_d[b, 0:512, :].rearrange("(t p) d -> p t d", p=P)
            )
            if T > 1:
                nc.scalar.dma_start(
                    ct_t[:, 1:T],
                    c_d[b, P:W, :].rearrange("(t p) d -> p t d", p=P),
                )
            nc.sync.dma_start(
                qt_t[:, 4:NT],
                q_d[b, 512:S, :].rearrange("(t p) d -> p t d", p=P),
            )
            nc.sync.dma_start(mk[:], mk_d[b])
            slots[b] = dict(qt_t=qt_t, ct_t=ct_t, mk=mk)

        def compute1(b):
            """norms, qn/cbt, qT/cT transposes, mm1+sigmoid."""
            T = ts[b]
            W = T * P
            NQC = (W + 511) // 512

            def ccol(kt):
                return 0 if kt == 0 else 4 + kt

            def qcol(t):
                return 1 + t if t < 4 else T + t

            NC_ = NT + T
            qt_t = slots[b]["qt_t"]
            ct_t = slots[b]["ct_t"]
            mk = slots[b]["mk"]

            ssq = mpool.tile([P, NT + NT], F32, tag="ssq")
            nrm = mpool.tile([P, NT + NT], F32, tag="nrm")
            inv = mpool.tile([P, NT + NT], F32, tag="inv")
            qnb = qbp.tile([P, NT, D], BF16, tag="qnb")
            cbt = cbp.tile([P, NT, D], BF16, tag="cbt")
            qT = tp.tile([P, ND, W], BF16, tag="qT")
            cT = tp.tile([P, ND, W], BF16, tag="cT")
            # sg[k', qb, kt, q_local]: one xbar transfer per qb gives the
            # scores row block [q, kt*P + k'] directly.
            sg = sgp.tile([P, T, T, P], BF16, tag="sg")

            def square(col, src):
                scr2 = spool.tile([P, D], BF16, tag="scr2")
                nc.scalar.activation(
                    scr2[:], src, AF.Square, accum_out=ssq[:, col : col + 1]
                )

            def transpose_tile(src, dst_T, t, evict_vec):
                pq = pst.tile([P, ND, P], BF16, tag="pt")
                for dch in range(ND):
                    nc.tensor.transpose(
                        pq[:, dch], src[:, dch * P : (dch + 1) * P], idb[:]
                    )
                nc.vector.tensor_copy(dst_T[:, :, t * P : (t + 1) * P], pq[:])

            # first half: c0 + q0..q3 norms (scalar Square/Sqrt block)
            square(ccol(0), ct_t[:, 0])
            for t in range(4):
                square(qcol(t), qt_t[:, t])
            nc.scalar.activation(nrm[:, 0:5], ssq[:, 0:5], AF.Sqrt)
            nc.vector.reciprocal(inv[:, 0:5], nrm[:, 0:5])

            nc.vector.tensor_copy(cbt[:, 0], ct_t[:, 0])
            for t in range(4):
                nc.vector.tensor_scalar_mul(
                    qnb[:, t], qt_t[:, t], inv[:, qcol(t) : qcol(t) + 1]
                )
            transpose_tile(cbt[:, 0], cT, 0, evict_vec=True)
            for t in range(min(4, T)):
                transpose_tile(qnb[:, t], qT, t, evict_vec=False)

            # second half: c1..cT-1 + q4..q7 norms
            for kt in range(1, T):
                square(ccol(kt), ct_t[:, kt])
            for t in range(4, NT):
                square(qcol(t), qt_t[:, t])
            if NC_ > 5:
                nc.scalar.activation(nrm[:, 5:NC_], ssq[:, 5:NC_], AF.Sqrt)
                nc.vector.reciprocal(inv[:, 5:NC_], nrm[:, 5:NC_])

            for kt in range(1, T):
                nc.vector.tensor_copy(cbt[:, kt], ct_t[:, kt])
                transpose_tile(cbt[:, kt], cT, kt, evict_vec=True)
            for t in range(4, NT):
                nc.vector.tensor_scalar_mul(
                    qnb[:, t], qt_t[:, t], inv[:, qcol(t) : qcol(t) + 1]
                )
                if t < T:
                    transpose_tile(qnb[:, t], qT, t, evict_vec=False)

            nc.gpsimd.dma_start(
                out_d[b, :, 0:D].rearrange("(t p) d -> p t d", p=P), qnb[:]
            )
            # zero fills for this slot (execute during compute)
            for qt in range(T, NT):
                nc.gpsimd.dma_start(sc_d[b, qt * P : (qt + 1) * P, :], zt[:])
                nc.gpsimd.dma_start(
                    out_d[b, qt * P : (qt + 1) * P, D : 2 * D], zt[:, 0:D]
                )

            # mm1: sigT[k, q] = sigmoid(inv_c[k] * (cT.T @ qT) + mask)
            for qc in range(NQC):
                wq = min(512, W - qc * 512)
                nqb = wq // P
                for kt in range(T):
                    acc = ps1.tile([P, 512], F32, tag="acc")
                    for dch in range(ND):
                        nc.tensor.matmul(
                            acc[:, 0:wq],
                            cT[:, dch, kt * P : (kt + 1) * P],
                            qT[:, dch, qc * 512 : qc * 512 + wq],
                            start=(dch == 0),
                            stop=(dch == ND - 1),
                        )
                    nc.scalar.activation(
                        sg[:, qc * 4 : qc * 4 + nqb, kt, :], acc[:, 0:wq],
                        AF.Sigmoid, bias=mk[:, kt : kt + 1],
                        scale=inv[:, ccol(kt) : ccol(kt) + 1],
                    )

            slots[b].update(T=T, W=W, cbt=cbt, sg=sg)

        def phase2(b):
            """per q-block: xbar score transpose, den, w, attended, writes."""
            st = slots.pop(b)
            T, W, mk, cbt, sg = st["T"], st["W"], st["mk"], st["cbt"], st["sg"]
            oq = nc.sync if b == BPC - 1 else nc.gpsimd
            aob = qbp.tile([P, T, D], BF16, tag="aob")
            for qb in range(T):
                so = opool.tile([P, T, P], BF16, tag="so")
                if W < S:
                    oq.dma_start(
                        sc_d[b, qb * P : (qb + 1) * P, W:S], zt[:, 0 : S - W]
                    )
                NKG = (T + 3) // 4
                dps = []
                for kg in range(NKG):
                    G = min(4, T - kg * 4)
                    pt = pst.tile([P, ND, P], BF16, tag="pt")
                    for j in range(G):
                        kt = kg * 4 + j
                        nc.tensor.transpose(pt[:, j], sg[:, qb, kt, :], idb[:])
                    # evict unscaled sigT^T; denominator rides along in
                    # the activation/tensor-scalar accumulator
                    dp = mpool.tile([P, 1], F32, tag=f"dp{kg}")
                    dps.append(dp)
                    nc.vector.tensor_scalar(
                        so[:, kg * 4 : kg * 4 + G, :], pt[:, 0:G],
                        1.0, None, op0=ALU.mult, op1=ALU.add,
                        accum_out=dp[:],
                    )

                att = ps2.tile([P, 512], F32, tag="att")
                for kt in range(T):
                    nc.tensor.matmul(
                        att[:], sg[:, qb, kt, :], cbt[:, kt],
                        start=(kt == 0), stop=(kt == T - 1),
                    )

                # w = qmask / max(den, 1)
                den = mpool.tile([P, 1], F32, tag="den")
                w = mpool.tile([P, 1], F32, tag="w")
                if NKG == 2:
                    nc.vector.tensor_add(den[:], dps[0][:], dps[1][:])
                else:
                    nc.vector.tensor_copy(den[:], dps[0][:])
                nc.vector.tensor_scalar_max(den[:], den[:], 1.0)
                nc.vector.reciprocal(w[:], den[:])
                nc.vector.tensor_mul(w[:], w[:], mk[:, NT + qb : NT + qb + 1])

                # scale scores in place, write out
                nc.vector.tensor_scalar_mul(so[:], so[:], w[:])
                oq.dma_start(sc_d[b, qb * P : (qb + 1) * P, 0:W], so[:])

                nc.vector.tensor_scalar_mul(aob[:, qb], att[:], w[:])

            oq.dma_start(
                out_d[b, 0:W, D : 2 * D].rearrange("(t p) d -> p t d", p=P),
                aob[:],
            )

        # sequential compute emission with input DMAs prefetched one slot
        # ahead (sync queue carries only input dispatches, so this only
        # reorders transfers, not compute).
        inputs(0)
        compute1(0)
        for b in range(BPC):
            if b + 1 < BPC:
                inputs(b + 1)
            phase2(b)
            if b + 1 < BPC:
                compute1(b + 1)


_NC_CACHE = {}


def _get_nc(ts):
    key = ("nc", ts)
    if key not in _NC_CACHE:
        _NC_CACHE[key] = build_kernel(ts)
    return _NC_CACHE[key]


def plan(length):
    """Sort batches by tile count desc, deal round-robin to cores.

    Returns (ts, order): ts[j] = baked tile count for slot j; order[j*NCORES+c]
    = batch index placed in slot j of core c.
    """
    length = np.asarray(length).astype(np.int64)
    T = np.ceil(length / P).astype(np.int64)
    order = np.argsort(-T, kind="stable")
    ts = tuple(int(T[order[j * NCORES]]) for j in range(BPC))
    return ts, order


def prep_inputs(context, query, length):
    context = np.ascontiguousarray(np.asarray(context, dtype=np.float32))
    query = np.ascontiguousarray(np.asarray(query, dtype=np.float32))
    length = np.asarray(length).astype(np.int64)
    ts, order = plan(length)

    iot = np.arange(S)
    keymask = iot[None, :] < length[:, None]                      # [B, S]
    kbH = np.where(keymask, np.float32(0.0), NEG).astype(np.float32)
    kbH = kbH.reshape(B, NT, P).transpose(0, 2, 1)
    qmH = keymask.astype(np.float32).reshape(B, NT, P).transpose(0, 2, 1)
    mkH = np.ascontiguousarray(np.concatenate([kbH, qmH], axis=2))
    idb = np.eye(P, dtype=ml_dtypes.bfloat16)

    in_maps = []
    for c in range(NCORES):
        bidx = [int(order[j * NCORES + c]) for j in range(BPC)]
        in_maps.append(
            {
                "query": np.ascontiguousarray(query[bidx]),
                "context": np.ascontiguousarray(context[bidx]),
                "masks": np.ascontiguousarray(mkH[bidx]),
                "identity": idb,
            }
        )
    return ts, order, in_maps


def kernel(context, query, length):
    ts, order, in_maps = prep_inputs(context, query, length)
    nc = _get_nc(ts)
    res = run_bass_kernel_spmd(nc, in_maps, list(range(NCORES)))
    _NC_CACHE["last_result"] = res

    out = np.empty((B, S, 2 * D), np.float32)
    scores = np.empty((B, S, S), np.float32)
    for c in range(NCORES):
        ro = np.asarray(res.results[c]["out"]).astype(np.float32)
        rs = np.asarray(res.results[c]["scores"]).astype(np.float32)
        for j in range(BPC):
            bi = int(order[j * NCORES + c])
            out[bi] = ro[j]
            scores[bi] = rs[j]
    return out, scores
